# revision 7
# baseline (speedup 1.0000x reference)
"""MinLSTM Trainium2 kernel — fp8 DoubleRow edition.

Full-input contract: kernel(**inputs) takes the complete (unsharded) numpy
inputs of the reference model and returns the full [B, T+1, H] float32 output.

Math (per batch b, channel h — identical to the reference's log-space scan,
computed in linear space; every quantity is positive so the linear recurrence
is numerically stable):
    a = x @ W_f ;  b = x @ W_i ;  c = x @ W_h        (biases are zero)
    f = sa/(sa+sb), i = 1-f   with sa = sigmoid(a), sb = sigmoid(b)
    g = max(c + 0.5, sigmoid(c))
    h_t = f_t * h_{t-1} + i_t * g_t,   h_{-1} = g(h_0)

Precision scheme (all matmuls fp8-e4m3 in DoubleRow mode, 0.5 cyc/row,
256-deep contraction per instruction):
    x8  = q8(x),  s16 = q8(16*(x - x8))          # dual-fp8 x
    a  = x8 @ q8(W_f) ;  b = x8 @ q8(W_i)        # gates tolerate fp8
    c  = x8 @ A1 + s16 @ B + x8 @ A2             # compensated h-projection
         A1 = q8(W_h), B = q8(W_h/16), A2 = q8(W_h - A1)
    s16@B recovers the x-quantization error (the /16 scale folds exactly:
    16*(x-x8) @ W_h/16); A2 recovers most of the W-quantization error via
    fp8 subnormals. Measured end-to-end l2 error ~9e-3 (budget 2e-2).

Sharding: 8 cores, core c -> (sample b = c//2, H-half hh = c%2, 256 channels).
Fully independent cores, no collectives. Host pre-transposes/pre-quantizes x;
host assembles the output (device writes h in f16, host upcasts).

Engine split per 512-wide T-chunk per h-tile:
    PE:   10 DoubleRow matmuls (a, b: 2 each; c: 6)
    Act:  sigmoid over packed [a;b] (1024), sigmoid over c (512)
    DVE:  w = sb/(sa+sb) fused custom op, f = 1-w (tensor_scalar 4x),
          h = tensor_tensor_scan(f, v), v-mult on even h-tiles
    Pool: g = max(c+0.5, sg) (scalar_tensor_tensor from PSUM),
          v-mult on odd h-tiles
"""

from contextlib import ExitStack

import numpy as np
import ml_dtypes

import concourse.bacc as bacc
import concourse.tile as tile
import concourse.mybir as mybir
from concourse.bass_utils import run_bass_kernel_spmd

# ---- fused custom DVE op: w = S1 * ~1/(S0+S1) ------------------------------
# x = sa+sb; nx = bitcast(~x) (exponent-flip seed); u = x*nx in [-4.5,-4];
# recip = nx * p1(u) with p1 a degree-1 minimax of 1/u on that interval
# (max rel err ~1.7e-3; the DVE pipeline is 8 sequential ALU stages, so the
# degree-2 poly + the *S1 fold does not fit); w = recip * sb.
import concourse.dve_ops as _dve_ops
from concourse.dve_spec import (Spec as _Spec, Src0 as _S0, Src1 as _S1,
                                C0 as _C0, C1 as _C1, C2 as _C2,
                                AluOp as _AluOp, Bin as _Bin, lower as _lower)
from concourse.dve_uop import DveOpSpec as _DveOpSpec
from concourse.dve_table_gen import dve_ver_for as _dve_ver_for

RECIP_CONSTS = {"s0": -0.05545928, "s1": -0.47140385, "imm2": 0.0}


def _register_dve_op(name, spec_builder):
    if name in _dve_ops._SUB_OPCODE_FOR_NAME:
        return next(o for o in _dve_ops.OPS if o.name == name)
    spec = spec_builder()
    row = _dve_ops._CUSTOM_DVE_ROW_BASE + len(_dve_ops.OPS)
    assert row < 0x20
    ver = _dve_ver_for("TRN2")
    sha = _DveOpSpec(name=name, opcode=row, uops=_lower(spec, ver=ver),
                     rd1_en=True).sha(ver)
    op = _dve_ops.DveOp(name, spec, subdim=False, uops_sha={ver: sha})
    _dve_ops.OPS.append(op)
    _dve_ops.CUSTOM_DVE_SPECS[name] = spec
    _dve_ops._SUB_OPCODE_FOR_NAME[name] = row
    return op


def _w_over_sum_spec():
    _x = _S0 + _S1
    _nx = _Bin(_AluOp.BITWISE_NOT, _x, _x)
    _u = _x * _nx

    def _ref(in0, in1, c0, c1, c2):
        x = (np.asarray(in0, np.float32) + np.asarray(in1, np.float32))
        x = x.astype(np.float32)
        nx = (~x.view(np.int32)).view(np.float32)
        u = x * nx
        return ((u * c0 + c1) * nx) * np.asarray(in1, np.float32)

    return _Spec(body=((_u * _C0 + _C1) * _nx) * _S1, reference=_ref)


_W_OVER_SUM_OP = _register_dve_op("W_OVER_SUM_ANT", _w_over_sum_spec)

F8 = mybir.dt.float8e4
F16 = mybir.dt.float16
F32 = mybir.dt.float32
AF = mybir.ActivationFunctionType
OP = mybir.AluOpType
PM = mybir.MatmulPerfMode

B, T, D, H = 4, 8192, 512, 512
NCORES = 8
HS = H // 2          # 256 channels per core
TC = 512             # T chunk width
NCH = T // TC        # 16 chunks
NKB = D // 256       # 2 DoubleRow contraction blocks
NHT = HS // 128      # 2 h-tiles per core
NW = 5               # stationary matrices: Wf, Wi, A1h, Bh, A2h

MM_MODE = "fp8"

_nc_cache = {}


def _build_nc(mm_mode=MM_MODE):
    assert mm_mode == "fp8"
    nc = bacc.Bacc("TRN2", target_bir_lowering=False, debug=False,
                   num_devices=NCORES)
    # x8/s16: [128, kb, i, T] with (p, kb, i) -> d = kb*256 + i*128 + p
    x8d = nc.dram_tensor("x8", [128, NKB, 2, T], F8, kind="ExternalInput")
    s16d = nc.dram_tensor("s16", [128, NKB, 2, T], F8, kind="ExternalInput")
    # stationaries: [128, group, i, 128], group = (mat, kb, ht)
    wd = nc.dram_tensor("w", [128, NW * NKB * NHT, 2, 128], F8,
                        kind="ExternalInput")
    aux = nc.dram_tensor("aux", [128, NHT], F32, kind="ExternalInput")
    out = nc.dram_tensor("out", [HS, T], F16, kind="ExternalOutput")

    def wg(mat, kb, ht):
        return (mat * NKB + kb) * NHT + ht

    with tile.TileContext(nc) as tc, ExitStack() as ctx:
        wpool = ctx.enter_context(tc.tile_pool(name="w", bufs=1))
        xpool = ctx.enter_context(tc.tile_pool(name="x", bufs=3))
        gpool = ctx.enter_context(tc.tile_pool(name="g", bufs=4))
        hpool = ctx.enter_context(tc.tile_pool(name="h", bufs=4))
        ppool = ctx.enter_context(tc.tile_pool(name="p", bufs=2, space="PSUM"))

        # weights + aux on the ACT HWDGE queue; x chunks on SP queue
        wt = wpool.tile([128, NW * NKB * NHT, 2, 128], F8, tag="wt", name="wt")
        nc.scalar.dma_start(wt[:], wd[:])
        auxt = wpool.tile([128, NHT], F32, tag="aux", name="auxt")
        nc.scalar.dma_start(auxt[:], aux[:])

        # chunk 0 split in half so the first matmul group starts on a
        # half-size x transfer
        chunks = [(0, TC // 2), (TC // 2, TC // 2)]
        chunks += [(k * TC, TC) for k in range(1, NCH)]

        carry = [None] * NHT
        cst = RECIP_CONSTS
        for ci, (t0, tw) in enumerate(chunks):
            tsl = slice(t0, t0 + tw)
            x8t = xpool.tile([128, NKB, 2, TC], F8, tag="x8", name="x8t")
            nc.sync.dma_start(x8t[:, :, :, :tw], x8d[:, :, :, tsl])
            s16t = xpool.tile([128, NKB, 2, TC], F8, tag="s16", name="s16t")
            nc.sync.dma_start(s16t[:, :, :, :tw], s16d[:, :, :, tsl])

            # PSUM: per-ht [f;i] pair tiles + one [c(ht0); c(ht1)] pair tile
            pfis = []
            pc2 = ppool.tile([128, 2, TC], F32, tag="pc2", bufs=2, name="pc2")
            hmm = [(2, x8t), (3, s16t), (4, x8t)]
            for ht in range(NHT):
                pfi = ppool.tile([128, 2, TC], F32, tag="pfi", bufs=2,
                                 name="pfi")
                pfis.append(pfi)
                for wi in range(2):
                    for kb in range(NKB):
                        nc.tensor.matmul(pfi[:, wi, :tw],
                                         wt[:, wg(wi, kb, ht)],
                                         x8t[:, kb, :, :tw],
                                         start=(kb == 0),
                                         stop=(kb == NKB - 1),
                                         perf_mode=PM.DoubleRow)
                # h-projection: x8@A1 + s16@B + x8@A2, 6 DoubleRow matmuls
                for mi, (mat, xt) in enumerate(hmm):
                    for kb in range(NKB):
                        nc.tensor.matmul(
                            pc2[:, ht, :tw], wt[:, wg(mat, kb, ht)],
                            xt[:, kb, :, :tw],
                            start=(mi == 0 and kb == 0),
                            stop=(mi == len(hmm) - 1 and kb == NKB - 1),
                            perf_mode=PM.DoubleRow)

            # sigmoids: one Act op per ht for [a;b], one merged op for both c
            sabm = gpool.tile([128, 2, 2, TC], F16, tag="sabm", name="sabm")
            for ht in range(NHT):
                nc.scalar.activation(sabm[:, ht, :, :tw], pfis[ht][:, :, :tw],
                                     AF.Sigmoid)
            sgm = gpool.tile([128, 2, TC], F16, tag="sgm", name="sgm")
            nc.scalar.activation(sgm[:, :, :tw], pc2[:, :, :tw], AF.Sigmoid)

            # f = sa/(sa+sb) for both ht in one fused custom op
            fm = gpool.tile([128, 2, TC], F16, tag="fm", name="fm")
            nc.vector._custom_dve(_W_OVER_SUM_OP, out=fm[:, :, :tw],
                                  in0=sabm[:, :, 1, :tw],
                                  in1=sabm[:, :, 0, :tw],
                                  s0=cst["s0"], s1=cst["s1"],
                                  imm2=cst["imm2"])
            # g = max(c+0.5, sigmoid(c)) for both ht (DVE: GpSimd can't PSUM)
            gm = gpool.tile([128, 2, TC], F16, tag="gm", name="gm")
            nc.vector.scalar_tensor_tensor(gm[:, :, :tw], pc2[:, :, :tw], 0.5,
                                           sgm[:, :, :tw], OP.add, OP.max)

            for ht in range(NHT):
                # vneg = f*g - g = -(1-f)*g on GpSimd (two plain TT ops —
                # Pool accepts neither PSUM operands nor TensorScalarPtr);
                # the scan then subtracts it
                fg = gpool.tile([128, TC], F16, tag=f"fg{ht}",
                                name=f"fg{ht}")[:, :tw]
                nc.gpsimd.tensor_tensor(fg, fm[:, ht, :tw], gm[:, ht, :tw],
                                        op=OP.mult)
                vneg = gpool.tile([128, TC], F16, tag=f"vn{ht}",
                                  name=f"vn{ht}")[:, :tw]
                nc.gpsimd.tensor_tensor(vneg, fg, gm[:, ht, :tw],
                                        op=OP.subtract)
                h = hpool.tile([128, TC], F16, tag=f"h{ht}",
                               name=f"h{ht}")[:, :tw]
                ini = auxt[:, ht:ht + 1] if ci == 0 else carry[ht]
                nc.vector.tensor_tensor_scan(h, fm[:, ht, :tw], vneg, ini,
                                             OP.mult, OP.subtract)
                carry[ht] = h[:, tw - 1:tw]
                nc.scalar.dma_start(out[ht * 128:(ht + 1) * 128, tsl], h)
    nc.compile()
    return nc


def _get_nc(mm_mode=MM_MODE):
    if mm_mode not in _nc_cache:
        _nc_cache[mm_mode] = _build_nc(mm_mode)
    return _nc_cache[mm_mode]


def _g_host(x):
    # exp(log_g(x)) of the reference, computed directly in fp32
    return np.where(x >= 0, x + 0.5, 1.0 / (1.0 + np.exp(-np.minimum(x, 0))))


E4 = ml_dtypes.float8_e4m3


def _q8(a):
    return np.asarray(a, E4)


def _pack_x(xT):
    # [D, T] -> [128, kb, i, T] with d = kb*256 + i*128 + p
    return np.ascontiguousarray(
        xT.reshape(NKB, 2, 128, T).transpose(2, 0, 1, 3))


def _pack_w(mats):
    # mats: list of NW arrays [D, HS] (fp8) -> [128, group, i, 128]
    wr = np.empty((128, NW * NKB * NHT, 2, 128), E4)
    for mat, m in enumerate(mats):
        r4 = m.reshape(NKB, 2, 128, NHT, 128)   # kb, i, p, ht, m
        for kb in range(NKB):
            for ht in range(NHT):
                g = (mat * NKB + kb) * NHT + ht
                wr[:, g] = r4[kb, :, :, ht].transpose(1, 0, 2)
    return wr


def _run(inputs, mm_mode=MM_MODE, trace=False):
    x = np.asarray(inputs["x"], np.float32)
    h_0 = np.asarray(inputs["h_0"], np.float32)
    W_f = np.asarray(inputs["W_f"], np.float32)
    b_f = np.asarray(inputs["b_f"], np.float32)
    W_i = np.asarray(inputs["W_i"], np.float32)
    b_i = np.asarray(inputs["b_i"], np.float32)
    W_h = np.asarray(inputs["W_h"], np.float32)
    b_h = np.asarray(inputs["b_h"], np.float32)
    assert (b_f == 0).all() and (b_i == 0).all() and (b_h == 0).all(), \
        "device program folds zero biases"

    g0 = _g_host(h_0[:, 0, :])  # [B, H]

    x8s, s16s = [], []
    for b in range(B):
        xT = np.ascontiguousarray(x[b].T)           # [D, T] f32
        x8 = _q8(xT)
        s16 = _q8(16.0 * (xT - x8.astype(np.float32)))
        x8s.append(_pack_x(x8))
        s16s.append(_pack_x(s16))

    in_maps = []
    for c in range(NCORES):
        b, hh = divmod(c, 2)
        hs = slice(hh * HS, (hh + 1) * HS)
        Wh = W_h[:, hs]
        A1 = _q8(Wh)
        A2 = _q8(Wh - A1.astype(np.float32))
        Bh = _q8(Wh / 16.0)
        wcat = _pack_w([_q8(W_f[:, hs]), _q8(W_i[:, hs]), A1, Bh, A2])
        auxa = np.ascontiguousarray(
            g0[b, hs].reshape(NHT, 128).T.astype(np.float32))
        in_maps.append({"x8": x8s[b], "s16": s16s[b], "w": wcat, "aux": auxa})

    nc = _get_nc(mm_mode)
    res = run_bass_kernel_spmd(nc, in_maps, core_ids=list(range(NCORES)),
                               trace=trace)

    out = np.empty((B, T + 1, H), np.float32)
    out[:, 0, :] = g0
    for c in range(NCORES):
        b, hh = divmod(c, 2)
        hs = slice(hh * HS, (hh + 1) * HS)
        out[b, 1:, hs] = res.results[c]["out"].T.astype(np.float32)
    return out, res


def kernel(**inputs):
    out, _ = _run(inputs)
    return out


# revision 9
# speedup vs baseline: 1.5591x; 1.5591x over previous
"""MinLSTM Trainium2 kernel — fp8 DoubleRow edition.

Full-input contract: kernel(**inputs) takes the complete (unsharded) numpy
inputs of the reference model and returns the full [B, T+1, H] float32 output.

Math (per batch b, channel h — identical to the reference's log-space scan,
computed in linear space; every quantity is positive so the linear recurrence
is numerically stable):
    a = x @ W_f ;  b = x @ W_i ;  c = x @ W_h        (biases are zero)
    f = sa/(sa+sb), i = 1-f   with sa = sigmoid(a), sb = sigmoid(b)
    g = max(c + 0.5, sigmoid(c))
    h_t = f_t * h_{t-1} + i_t * g_t,   h_{-1} = g(h_0)

Precision scheme (all matmuls fp8-e4m3 in DoubleRow mode, 0.5 cyc/row,
256-deep contraction per instruction):
    x8  = q8(x),  s16 = q8(16*(x - x8))          # dual-fp8 x
    a  = x8 @ q8(W_f) ;  b = x8 @ q8(W_i)        # gates tolerate fp8
    c  = x8 @ A1 + s16 @ B + x8 @ A2             # compensated h-projection
         A1 = q8(W_h), B = q8(W_h/16), A2 = q8(W_h - A1)
    s16@B recovers the x-quantization error (the /16 scale folds exactly:
    16*(x-x8) @ W_h/16); A2 recovers most of the W-quantization error via
    fp8 subnormals. Measured end-to-end l2 error ~9e-3 (budget 2e-2).

Sharding: 8 cores, core c -> (sample b = c//2, H-half hh = c%2, 256 channels).
Fully independent cores, no collectives. Host pre-transposes/pre-quantizes x;
host assembles the output (device writes h in f16, host upcasts).

Engine split per 512-wide T-chunk per h-tile:
    PE:   10 DoubleRow matmuls (a, b: 2 each; c: 6)
    Act:  sigmoid over packed [a;b] (1024), sigmoid over c (512)
    DVE:  w = sb/(sa+sb) fused custom op, f = 1-w (tensor_scalar 4x),
          h = tensor_tensor_scan(f, v), v-mult on even h-tiles
    Pool: g = max(c+0.5, sg) (scalar_tensor_tensor from PSUM),
          v-mult on odd h-tiles
"""

from contextlib import ExitStack

import numpy as np
import ml_dtypes

import concourse.bacc as bacc
import concourse.tile as tile
import concourse.mybir as mybir
from concourse.bass_utils import run_bass_kernel_spmd

# ---- fused custom DVE op: w = S1 * ~1/(S0+S1) ------------------------------
# x = sa+sb; nx = bitcast(~x) (exponent-flip seed); u = x*nx in [-4.5,-4];
# recip = nx * p1(u) with p1 a degree-1 minimax of 1/u on that interval
# (max rel err ~1.7e-3; the DVE pipeline is 8 sequential ALU stages, so the
# degree-2 poly + the *S1 fold does not fit); w = recip * sb.
import concourse.dve_ops as _dve_ops
from concourse.dve_spec import (Spec as _Spec, Src0 as _S0, Src1 as _S1,
                                C0 as _C0, C1 as _C1, C2 as _C2,
                                AluOp as _AluOp, Bin as _Bin, lower as _lower)
from concourse.dve_uop import DveOpSpec as _DveOpSpec
from concourse.dve_table_gen import dve_ver_for as _dve_ver_for

RECIP_CONSTS = {"s0": -0.05545928, "s1": -0.47140385, "imm2": 0.0}


def _register_dve_op(name, spec_builder):
    if name in _dve_ops._SUB_OPCODE_FOR_NAME:
        return next(o for o in _dve_ops.OPS if o.name == name)
    spec = spec_builder()
    row = _dve_ops._CUSTOM_DVE_ROW_BASE + len(_dve_ops.OPS)
    assert row < 0x20
    ver = _dve_ver_for("TRN2")
    sha = _DveOpSpec(name=name, opcode=row, uops=_lower(spec, ver=ver),
                     rd1_en=True).sha(ver)
    op = _dve_ops.DveOp(name, spec, subdim=False, uops_sha={ver: sha})
    _dve_ops.OPS.append(op)
    _dve_ops.CUSTOM_DVE_SPECS[name] = spec
    _dve_ops._SUB_OPCODE_FOR_NAME[name] = row
    return op


def _w_over_sum_spec():
    _x = _S0 + _S1
    _nx = _Bin(_AluOp.BITWISE_NOT, _x, _x)
    _u = _x * _nx

    def _ref(in0, in1, c0, c1, c2):
        x = (np.asarray(in0, np.float32) + np.asarray(in1, np.float32))
        x = x.astype(np.float32)
        nx = (~x.view(np.int32)).view(np.float32)
        u = x * nx
        return ((u * c0 + c1) * nx) * np.asarray(in1, np.float32)

    return _Spec(body=((_u * _C0 + _C1) * _nx) * _S1, reference=_ref)


_W_OVER_SUM_OP = _register_dve_op("W_OVER_SUM_ANT", _w_over_sum_spec)

F8 = mybir.dt.float8e4
F16 = mybir.dt.float16
F32 = mybir.dt.float32
AF = mybir.ActivationFunctionType
OP = mybir.AluOpType
PM = mybir.MatmulPerfMode

B, T, D, H = 4, 8192, 512, 512
NCORES = 8
HS = H // 2          # 256 channels per core
TC = 512             # T chunk width
NCH = T // TC        # 16 chunks
NKB = D // 256       # 2 DoubleRow contraction blocks
NHT = HS // 128      # 2 h-tiles per core
NW = 5               # stationary matrices: Wf, Wi, A1h, Bh, A2h

MM_MODE = "fp8"

_nc_cache = {}


def _build_nc(mm_mode=MM_MODE):
    assert mm_mode == "fp8"
    nc = bacc.Bacc("TRN2", target_bir_lowering=False, debug=False,
                   num_devices=NCORES)
    # x8/s16: [128, kb, i, T] with (p, kb, i) -> d = kb*256 + i*128 + p
    x8d = nc.dram_tensor("x8", [128, NKB, 2, T], F8, kind="ExternalInput")
    s16d = nc.dram_tensor("s16", [128, NKB, 2, T], F8, kind="ExternalInput")
    # stationaries: [128, group, i, 128], group = (mat, kb, ht)
    wd = nc.dram_tensor("w", [128, NW * NKB * NHT, 2, 128], F8,
                        kind="ExternalInput")
    aux = nc.dram_tensor("aux", [128, NHT], F32, kind="ExternalInput")
    out = nc.dram_tensor("out", [HS, T], F16, kind="ExternalOutput")

    def wg(mat, kb, ht):
        return (mat * NKB + kb) * NHT + ht

    with tile.TileContext(nc) as tc, ExitStack() as ctx:
        wpool = ctx.enter_context(tc.tile_pool(name="w", bufs=1))
        xpool = ctx.enter_context(tc.tile_pool(name="x", bufs=3))
        gpool = ctx.enter_context(tc.tile_pool(name="g", bufs=4))
        hpool = ctx.enter_context(tc.tile_pool(name="h", bufs=4))
        ppool = ctx.enter_context(tc.tile_pool(name="p", bufs=2, space="PSUM"))

        # weights + aux on the ACT HWDGE queue; x chunks on SP queue
        wt = wpool.tile([128, NW * NKB * NHT, 2, 128], F8, tag="wt", name="wt")
        nc.scalar.dma_start(wt[:], wd[:])
        auxt = wpool.tile([128, NHT], F32, tag="aux", name="auxt")
        nc.scalar.dma_start(auxt[:], aux[:])

        # chunk 0 split in half so the first matmul group starts on a
        # half-size x transfer
        chunks = [(0, TC // 2), (TC // 2, TC // 2)]
        chunks += [(k * TC, TC) for k in range(1, NCH)]

        carry = [None] * NHT
        cst = RECIP_CONSTS
        pend = None

        def emit_scans(ci, fm, vm, tw, tsl):
            for ht in range(NHT):
                h = hpool.tile([128, TC], F16, tag=f"h{ht}",
                               name=f"h{ht}")[:, :tw]
                ini = auxt[:, ht:ht + 1] if ci == 0 else carry[ht]
                nc.vector.tensor_tensor_scan(h, fm[:, ht, :tw],
                                             vm[:, ht, :tw], ini,
                                             OP.mult, OP.add)
                carry[ht] = h[:, tw - 1:tw]
                nc.sync.dma_start(out[ht * 128:(ht + 1) * 128, tsl], h)

        for ci, (t0, tw) in enumerate(chunks):
            tsl = slice(t0, t0 + tw)
            x8t = xpool.tile([128, NKB, 2, TC], F8, tag="x8", name="x8t")
            nc.sync.dma_start(x8t[:, :, :, :tw], x8d[:, :, :, tsl])
            s16t = xpool.tile([128, NKB, 2, TC], F8, tag="s16", name="s16t")
            nc.sync.dma_start(s16t[:, :, :, :tw], s16d[:, :, :, tsl])

            # PSUM: per-ht [f;i] pair tiles + one [c(ht0); c(ht1)] pair tile
            pfis = []
            pc2 = ppool.tile([128, 2, TC], F32, tag="pc2", bufs=2, name="pc2")
            hmm = [(2, x8t), (3, s16t), (4, x8t)]
            for ht in range(NHT):
                pfi = ppool.tile([128, 2, TC], F32, tag="pfi", bufs=2,
                                 name="pfi")
                pfis.append(pfi)
                for wi in range(2):
                    for kb in range(NKB):
                        nc.tensor.matmul(pfi[:, wi, :tw],
                                         wt[:, wg(wi, kb, ht)],
                                         x8t[:, kb, :, :tw],
                                         start=(kb == 0),
                                         stop=(kb == NKB - 1),
                                         perf_mode=PM.DoubleRow)
                # h-projection: x8@A1 + s16@B + x8@A2, 6 DoubleRow matmuls
                for mi, (mat, xt) in enumerate(hmm):
                    for kb in range(NKB):
                        nc.tensor.matmul(
                            pc2[:, ht, :tw], wt[:, wg(mat, kb, ht)],
                            xt[:, kb, :, :tw],
                            start=(mi == 0 and kb == 0),
                            stop=(mi == len(hmm) - 1 and kb == NKB - 1),
                            perf_mode=PM.DoubleRow)

            # sigmoids: one Act op per ht for [a;b], one merged op for both c
            sabm = gpool.tile([128, 2, 2, TC], F16, tag="sabm", name="sabm")
            for ht in range(NHT):
                nc.scalar.activation(sabm[:, ht, :, :tw], pfis[ht][:, :, :tw],
                                     AF.Sigmoid)
            sgm = gpool.tile([128, 2, TC], F16, tag="sgm", name="sgm")
            nc.scalar.activation(sgm[:, :, :tw], pc2[:, :, :tw], AF.Sigmoid)

            # w = sb/(sa+sb) = 1-f for both ht in one fused custom op
            wm = gpool.tile([128, 2, TC], F16, tag="wm", name="wm")
            nc.vector._custom_dve(_W_OVER_SUM_OP, out=wm[:, :, :tw],
                                  in0=sabm[:, :, 0, :tw],
                                  in1=sabm[:, :, 1, :tw],
                                  s0=cst["s0"], s1=cst["s1"],
                                  imm2=cst["imm2"])
            # f = 1-w (tensor_scalar runs in the 4x DVE mode)
            fm = gpool.tile([128, 2, TC], F16, tag="fm", name="fm")
            nc.vector.tensor_scalar(fm[:, :, :tw], wm[:, :, :tw], -1.0, 1.0,
                                    OP.mult, OP.add)
            # g = max(c+0.5, sigmoid(c)) for both ht (DVE: GpSimd can't PSUM)
            gm = gpool.tile([128, 2, TC], F16, tag="gm", name="gm")
            nc.vector.scalar_tensor_tensor(gm[:, :, :tw], pc2[:, :, :tw], 0.5,
                                           sgm[:, :, :tw], OP.add, OP.max)
            # v = w*g on GpSimd, one merged op for both ht
            vm = gpool.tile([128, 2, TC], F16, tag="vm", name="vm")
            nc.gpsimd.tensor_tensor(vm[:, :, :tw], wm[:, :, :tw],
                                    gm[:, :, :tw], op=OP.mult)

            # software pipelining: the sequential scan (and the h writeback)
            # of chunk k is emitted during chunk k+1, so the per-engine
            # instruction queues never block on the scan chain
            if pend is not None:
                emit_scans(*pend)
            pend = (ci, fm, vm, tw, tsl)
        emit_scans(*pend)
    nc.compile()
    return nc


def _get_nc(mm_mode=MM_MODE):
    if mm_mode not in _nc_cache:
        _nc_cache[mm_mode] = _build_nc(mm_mode)
    return _nc_cache[mm_mode]


def _g_host(x):
    # exp(log_g(x)) of the reference, computed directly in fp32
    return np.where(x >= 0, x + 0.5, 1.0 / (1.0 + np.exp(-np.minimum(x, 0))))


E4 = ml_dtypes.float8_e4m3


def _q8(a):
    return np.asarray(a, E4)


def _pack_x(xT):
    # [D, T] -> [128, kb, i, T] with d = kb*256 + i*128 + p
    return np.ascontiguousarray(
        xT.reshape(NKB, 2, 128, T).transpose(2, 0, 1, 3))


def _pack_w(mats):
    # mats: list of NW arrays [D, HS] (fp8) -> [128, group, i, 128]
    wr = np.empty((128, NW * NKB * NHT, 2, 128), E4)
    for mat, m in enumerate(mats):
        r4 = m.reshape(NKB, 2, 128, NHT, 128)   # kb, i, p, ht, m
        for kb in range(NKB):
            for ht in range(NHT):
                g = (mat * NKB + kb) * NHT + ht
                wr[:, g] = r4[kb, :, :, ht].transpose(1, 0, 2)
    return wr


def _run(inputs, mm_mode=MM_MODE, trace=False):
    x = np.asarray(inputs["x"], np.float32)
    h_0 = np.asarray(inputs["h_0"], np.float32)
    W_f = np.asarray(inputs["W_f"], np.float32)
    b_f = np.asarray(inputs["b_f"], np.float32)
    W_i = np.asarray(inputs["W_i"], np.float32)
    b_i = np.asarray(inputs["b_i"], np.float32)
    W_h = np.asarray(inputs["W_h"], np.float32)
    b_h = np.asarray(inputs["b_h"], np.float32)
    assert (b_f == 0).all() and (b_i == 0).all() and (b_h == 0).all(), \
        "device program folds zero biases"

    g0 = _g_host(h_0[:, 0, :])  # [B, H]

    x8s, s16s = [], []
    for b in range(B):
        xT = np.ascontiguousarray(x[b].T)           # [D, T] f32
        x8 = _q8(xT)
        s16 = _q8(16.0 * (xT - x8.astype(np.float32)))
        x8s.append(_pack_x(x8))
        s16s.append(_pack_x(s16))

    in_maps = []
    for c in range(NCORES):
        b, hh = divmod(c, 2)
        hs = slice(hh * HS, (hh + 1) * HS)
        Wh = W_h[:, hs]
        A1 = _q8(Wh)
        A2 = _q8(Wh - A1.astype(np.float32))
        Bh = _q8(Wh / 16.0)
        wcat = _pack_w([_q8(W_f[:, hs]), _q8(W_i[:, hs]), A1, Bh, A2])
        auxa = np.ascontiguousarray(
            g0[b, hs].reshape(NHT, 128).T.astype(np.float32))
        in_maps.append({"x8": x8s[b], "s16": s16s[b], "w": wcat, "aux": auxa})

    nc = _get_nc(mm_mode)
    res = run_bass_kernel_spmd(nc, in_maps, core_ids=list(range(NCORES)),
                               trace=trace)

    out = np.empty((B, T + 1, H), np.float32)
    out[:, 0, :] = g0
    for c in range(NCORES):
        b, hh = divmod(c, 2)
        hs = slice(hh * HS, (hh + 1) * HS)
        out[b, 1:, hs] = res.results[c]["out"].T.astype(np.float32)
    return out, res


def kernel(**inputs):
    out, _ = _run(inputs)
    return out


# revision 36
# speedup vs baseline: 1.6625x; 1.0664x over previous
"""MinLSTM Trainium2 kernel — fp8 DoubleRow edition.

Full-input contract: kernel(**inputs) takes the complete (unsharded) numpy
inputs of the reference model and returns the full [B, T+1, H] float32 output.

Math (per batch b, channel h — identical to the reference's log-space scan,
computed in linear space; every quantity is positive so the linear recurrence
is numerically stable):
    a = x @ W_f ;  b = x @ W_i ;  c = x @ W_h        (biases are zero)
    f = sa/(sa+sb), i = 1-f   with sa = sigmoid(a), sb = sigmoid(b)
    g = max(c + 0.5, sigmoid(c))
    h_t = f_t * h_{t-1} + i_t * g_t,   h_{-1} = g(h_0)

Precision scheme (all matmuls fp8-e4m3 in DoubleRow mode, 0.5 cyc/row,
256-deep contraction per instruction):
    x8  = q8(x),  s16 = q8(16*(x - x8))          # dual-fp8 x
    a  = x8 @ q8(W_f) ;  b = x8 @ q8(W_i)        # gates tolerate fp8
    c  = x8 @ A1 + s16 @ B + x8 @ A2             # compensated h-projection
         A1 = q8(W_h), B = q8(W_h/16), A2 = q8(W_h - A1)
    s16@B recovers the x-quantization error (the /16 scale folds exactly:
    16*(x-x8) @ W_h/16); A2 recovers most of the W-quantization error via
    fp8 subnormals. Measured end-to-end l2 error ~9e-3 (budget 2e-2).

Sharding: 8 cores, core c -> (sample b = c//2, H-half hh = c%2, 256 channels).
Fully independent cores, no collectives. Host pre-transposes/pre-quantizes x;
host assembles the output (device writes h in f16, host upcasts).

Engine split per 512-wide T-chunk per h-tile:
    PE:   10 DoubleRow matmuls (a, b: 2 each; c: 6)
    Act:  sigmoid over packed [a;b] (1024), sigmoid over c (512)
    DVE:  w = sb/(sa+sb) fused custom op, f = 1-w (tensor_scalar 4x),
          h = tensor_tensor_scan(f, v), v-mult on even h-tiles
    Pool: g = max(c+0.5, sg) (scalar_tensor_tensor from PSUM),
          v-mult on odd h-tiles
"""

from contextlib import ExitStack

import numpy as np
import ml_dtypes

import concourse.bacc as bacc
import concourse.tile as tile
import concourse.mybir as mybir
from concourse.bass_utils import run_bass_kernel_spmd

# ---- fused custom DVE op: w = S1 * ~1/(S0+S1) ------------------------------
# x = sa+sb; nx = bitcast(~x) (exponent-flip seed); u = x*nx in [-4.5,-4];
# recip = nx * p1(u) with p1 a degree-1 minimax of 1/u on that interval
# (max rel err ~1.7e-3; the DVE pipeline is 8 sequential ALU stages, so the
# degree-2 poly + the *S1 fold does not fit); w = recip * sb.
import concourse.dve_ops as _dve_ops
from concourse.dve_spec import (Spec as _Spec, Src0 as _S0, Src1 as _S1,
                                C0 as _C0, C1 as _C1, C2 as _C2,
                                AluOp as _AluOp, Bin as _Bin, lower as _lower)
from concourse.dve_uop import DveOpSpec as _DveOpSpec
from concourse.dve_table_gen import dve_ver_for as _dve_ver_for

RECIP_CONSTS = {"s0": -0.05545928, "s1": -0.47140385, "imm2": 0.0}


def _register_dve_op(name, spec_builder):
    if name in _dve_ops._SUB_OPCODE_FOR_NAME:
        return next(o for o in _dve_ops.OPS if o.name == name)
    spec = spec_builder()
    row = _dve_ops._CUSTOM_DVE_ROW_BASE + len(_dve_ops.OPS)
    assert row < 0x20
    ver = _dve_ver_for("TRN2")
    sha = _DveOpSpec(name=name, opcode=row, uops=_lower(spec, ver=ver),
                     rd1_en=True).sha(ver)
    op = _dve_ops.DveOp(name, spec, subdim=False, uops_sha={ver: sha})
    _dve_ops.OPS.append(op)
    _dve_ops.CUSTOM_DVE_SPECS[name] = spec
    _dve_ops._SUB_OPCODE_FOR_NAME[name] = row
    return op


def _w_over_sum_spec():
    _x = _S0 + _S1
    _nx = _Bin(_AluOp.BITWISE_NOT, _x, _x)
    _u = _x * _nx

    def _ref(in0, in1, c0, c1, c2):
        x = (np.asarray(in0, np.float32) + np.asarray(in1, np.float32))
        x = x.astype(np.float32)
        nx = (~x.view(np.int32)).view(np.float32)
        u = x * nx
        return ((u * c0 + c1) * nx) * np.asarray(in1, np.float32)

    return _Spec(body=((_u * _C0 + _C1) * _nx) * _S1, reference=_ref)


_W_OVER_SUM_OP = _register_dve_op("W_OVER_SUM_ANT", _w_over_sum_spec)

F8 = mybir.dt.float8e4
F16 = mybir.dt.float16
F32 = mybir.dt.float32
AF = mybir.ActivationFunctionType
OP = mybir.AluOpType
PM = mybir.MatmulPerfMode

B, T, D, H = 4, 8192, 512, 512
NCORES = 8
HS = H // 2          # 256 channels per core
TC = 512             # T chunk width
NCH = T // TC        # 16 chunks
NKB = D // 256       # 2 DoubleRow contraction blocks
NHT = HS // 128      # 2 h-tiles per core
NW = 5               # stationary matrices: Wf, Wi, A1h, Bh, A2h

MM_MODE = "fp8"

_nc_cache = {}


def _build_nc(mm_mode=MM_MODE):
    assert mm_mode == "fp8"
    nc = bacc.Bacc("TRN2", target_bir_lowering=False, debug=False,
                   num_devices=NCORES)
    # x8/s16: [128, kb, i, T] with (p, kb, i) -> d = kb*256 + i*128 + p
    x8d = nc.dram_tensor("x8", [128, NKB, 2, T], F8, kind="ExternalInput")
    s16d = nc.dram_tensor("s16", [128, NKB, 2, T], F8, kind="ExternalInput")
    # contiguous copies of the two chunk-0 halves (a T-slice of x8d has
    # 256B contiguous runs and pays the sub-512B DMA latency penalty; the
    # warmup transfers sit on the critical path)
    xh0 = nc.dram_tensor("xh0", [128, 2, NKB, 2, TC // 2], F8,
                         kind="ExternalInput")
    sh0 = nc.dram_tensor("sh0", [128, 2, NKB, 2, TC // 2], F8,
                         kind="ExternalInput")
    # stationaries: [128, group, i, 128], group = (mat, kb, ht)
    wd = nc.dram_tensor("w", [128, NW * NKB * NHT, 2, 128], F8,
                        kind="ExternalInput")
    aux = nc.dram_tensor("aux", [128, NHT], F32, kind="ExternalInput")
    # out[p, ht, t] = h-channel (ht*128 + p) at step t — (p, ht, t) order
    # matches the packed on-chip h tile so one DMA covers both h-tiles
    out = nc.dram_tensor("out", [128, NHT, T], F16, kind="ExternalOutput")

    def wg(mat, kb, ht):
        return (mat * NKB + kb) * NHT + ht

    with tile.TileContext(nc) as tc, ExitStack() as ctx:
        wpool = ctx.enter_context(tc.tile_pool(name="w", bufs=1))
        xpool = ctx.enter_context(tc.tile_pool(name="x", bufs=3))
        gpool = ctx.enter_context(tc.tile_pool(name="g", bufs=4))
        hpool = ctx.enter_context(tc.tile_pool(name="h", bufs=4))
        ppool = ctx.enter_context(tc.tile_pool(name="p", bufs=2, space="PSUM"))

        # weights on the SP queue, f/i matrices first — their (smaller)
        # transfer plus the first x half-chunk gates the first matmuls; the
        # h-projection matrices follow and land before the first c-matmuls.
        # aux rides the Act queue (its SP slot would delay the x chain by
        # one 650ns DMA-issue slot).
        NG_FI = 2 * NKB * NHT
        NG_A1 = 3 * NKB * NHT
        wt = wpool.tile([128, NW * NKB * NHT, 2, 128], F8, tag="wt", name="wt")
        nc.sync.dma_start(wt[:, :NG_FI], wd[:, :NG_FI])
        auxt = wpool.tile([128, NHT], F32, tag="aux", name="auxt")
        nc.gpsimd.dma_start(auxt[:], aux[:])
        ones = wpool.tile([128, TC], F16, tag="ones", name="ones")
        nc.gpsimd.memset(ones[:], 1.0)

        # chunk 0 split in half so the pipeline fills on half-size work;
        # the last chunk split in half so it drains on half-size work
        chunks = [(0, TC // 2), (TC // 2, TC // 2)]
        chunks += [(k * TC, TC) for k in range(1, NCH - 1)]
        chunks += [((NCH - 1) * TC, TC // 2),
                   ((NCH - 1) * TC + TC // 2, TC // 2)]

        carry = [None] * NHT
        cst = RECIP_CONSTS
        pend = None

        def emit_scans(ci, fm, vm, tw, tsl):
            h2 = hpool.tile([128, NHT, TC], F16, tag="h2", name="h2")
            for ht in range(NHT):
                ini = auxt[:, ht:ht + 1] if ci == 0 else carry[ht]
                nc.vector.tensor_tensor_scan(h2[:, ht, :tw], fm[:, ht, :tw],
                                             vm[:, ht, :tw], ini,
                                             OP.mult, OP.add)
                carry[ht] = h2[:, ht, tw - 1:tw]
            # one DMA for both h-tiles; the second-to-last goes on the (by
            # then idle) Act queue so the two drain writebacks overlap
            q = nc.scalar if ci == len(chunks) - 2 else nc.sync
            q.dma_start(out[:, :, tsl], h2[:, :, :tw])

        for ci, (t0, tw) in enumerate(chunks):
            tsl = slice(t0, t0 + tw)
            x8t = xpool.tile([128, NKB, 2, tw], F8, tag="x8", name="x8t")
            s16t = xpool.tile([128, NKB, 2, tw], F8, tag="s16", name="s16t")
            if ci < 2:
                nc.sync.dma_start(x8t[:], xh0[:, ci])
                nc.sync.dma_start(s16t[:], sh0[:, ci])
                if ci == 0:
                    # A1 right after the first x half-chunk (the warmup
                    # chunks' c-projection uses A1 only); B/A2 follow
                    nc.sync.dma_start(wt[:, NG_FI:NG_A1], wd[:, NG_FI:NG_A1])
                else:
                    nc.sync.dma_start(wt[:, NG_A1:], wd[:, NG_A1:])
            else:
                nc.sync.dma_start(x8t[:], x8d[:, :, :, tsl])
                nc.sync.dma_start(s16t[:], s16d[:, :, :, tsl])

            # PSUM: per-ht [f;i] pair tiles + one [c(ht0); c(ht1)] pair tile
            pfis = []
            pc2 = ppool.tile([128, 2, TC], F32, tag="pc2", bufs=2, name="pc2")
            # warmup chunks skip the B/A2 compensation terms: their weights
            # arrive later, and the ~3.6% c-error over the first 512 of 8192
            # steps decays through the forget gate (small l2 impact)
            hmm = [(2, x8t)] if ci < 2 else [(2, x8t), (3, s16t), (4, x8t)]
            # all f/i matmuls first (both ht) so the sigmoid chain — which
            # feeds the critical DVE stream — starts as early as possible
            for ht in range(NHT):
                pfi = ppool.tile([128, 2, TC], F32, tag="pfi", bufs=2,
                                 name="pfi")
                pfis.append(pfi)
                for wi in range(2):
                    for kb in range(NKB):
                        nc.tensor.matmul(pfi[:, wi, :tw],
                                         wt[:, wg(wi, kb, ht)],
                                         x8t[:, kb, :, :tw],
                                         start=(kb == 0),
                                         stop=(kb == NKB - 1),
                                         perf_mode=PM.DoubleRow)
            # h-projection: x8@A1 + s16@B + x8@A2, 6 DoubleRow matmuls per ht
            for ht in range(NHT):
                for mi, (mat, xt) in enumerate(hmm):
                    for kb in range(NKB):
                        nc.tensor.matmul(
                            pc2[:, ht, :tw], wt[:, wg(mat, kb, ht)],
                            xt[:, kb, :, :tw],
                            start=(mi == 0 and kb == 0),
                            stop=(mi == len(hmm) - 1 and kb == NKB - 1),
                            perf_mode=PM.DoubleRow)

            # sigmoids: one Act op per ht for [a;b], one merged op for both c
            sabm = gpool.tile([128, 2, 2, TC], F16, tag="sabm", name="sabm")
            for ht in range(NHT):
                nc.scalar.activation(sabm[:, ht, :, :tw], pfis[ht][:, :, :tw],
                                     AF.Sigmoid)
            sgm = gpool.tile([128, 2, TC], F16, tag="sgm", name="sgm")
            nc.scalar.activation(sgm[:, :, :tw], pc2[:, :, :tw], AF.Sigmoid)

            # w = sb/(sa+sb) = 1-f in one fused custom op; f = 1-w runs in
            # the 4x tensor_scalar mode. Merged over both ht in steady state;
            # per-ht during warmup so DVE starts after a single sigmoid.
            wm = gpool.tile([128, 2, TC], F16, tag="wm", name="wm")
            fm = gpool.tile([128, 2, TC], F16, tag="fm", name="fm")
            hts = [(slice(ht, ht + 1)) for ht in range(NHT)] if ci < 2 \
                else [slice(None)]
            for hsl in hts:
                nc.vector._custom_dve(_W_OVER_SUM_OP, out=wm[:, hsl, :tw],
                                      in0=sabm[:, hsl, 0, :tw],
                                      in1=sabm[:, hsl, 1, :tw],
                                      s0=cst["s0"], s1=cst["s1"],
                                      imm2=cst["imm2"])
            if ci < 2:
                for ht in range(NHT):
                    nc.vector.tensor_scalar(fm[:, ht, :tw], wm[:, ht, :tw],
                                            -1.0, 1.0, OP.mult, OP.add)
            else:
                # ht0's f on GpSimd (it has slack), ht1's in the 4x TS mode
                nc.gpsimd.tensor_tensor(fm[:, 0, :tw], ones[:, :tw],
                                        wm[:, 0, :tw], op=OP.subtract)
                nc.vector.tensor_scalar(fm[:, 1, :tw], wm[:, 1, :tw],
                                        -1.0, 1.0, OP.mult, OP.add)
            # g = max(c+0.5, sigmoid(c)) for both ht (DVE: GpSimd can't PSUM)
            gm = gpool.tile([128, 2, TC], F16, tag="gm", name="gm")
            nc.vector.scalar_tensor_tensor(gm[:, :, :tw], pc2[:, :, :tw], 0.5,
                                           sgm[:, :, :tw], OP.add, OP.max)
            # v = w*g, one merged op for both ht — on GpSimd in steady state;
            # the two drain chunks use DVE (2x f16 mode) to shorten the tail
            vm = gpool.tile([128, 2, TC], F16, tag="vm", name="vm")
            veng = nc.vector if ci >= len(chunks) - 2 else nc.gpsimd
            veng.tensor_tensor(vm[:, :, :tw], wm[:, :, :tw],
                               gm[:, :, :tw], op=OP.mult)

            # software pipelining: the sequential scan (and the h writeback)
            # of chunk k is emitted during chunk k+1, so the per-engine
            # instruction queues never block on the scan chain
            if pend is not None:
                emit_scans(*pend)
            pend = (ci, fm, vm, tw, tsl)
        emit_scans(*pend)
    nc.compile()
    return nc


def _get_nc(mm_mode=MM_MODE):
    if mm_mode not in _nc_cache:
        _nc_cache[mm_mode] = _build_nc(mm_mode)
    return _nc_cache[mm_mode]


def _g_host(x):
    # exp(log_g(x)) of the reference, computed directly in fp32
    return np.where(x >= 0, x + 0.5, 1.0 / (1.0 + np.exp(-np.minimum(x, 0))))


E4 = ml_dtypes.float8_e4m3


def _q8(a):
    return np.asarray(a, E4)


def _pack_x(xT):
    # [D, T] -> [128, kb, i, T] with d = kb*256 + i*128 + p
    return np.ascontiguousarray(
        xT.reshape(NKB, 2, 128, T).transpose(2, 0, 1, 3))


def _pack_w(mats):
    # mats: list of NW arrays [D, HS] (fp8) -> [128, group, i, 128]
    wr = np.empty((128, NW * NKB * NHT, 2, 128), E4)
    for mat, m in enumerate(mats):
        r4 = m.reshape(NKB, 2, 128, NHT, 128)   # kb, i, p, ht, m
        for kb in range(NKB):
            for ht in range(NHT):
                g = (mat * NKB + kb) * NHT + ht
                wr[:, g] = r4[kb, :, :, ht].transpose(1, 0, 2)
    return wr


def _run(inputs, mm_mode=MM_MODE, trace=False):
    x = np.asarray(inputs["x"], np.float32)
    h_0 = np.asarray(inputs["h_0"], np.float32)
    W_f = np.asarray(inputs["W_f"], np.float32)
    b_f = np.asarray(inputs["b_f"], np.float32)
    W_i = np.asarray(inputs["W_i"], np.float32)
    b_i = np.asarray(inputs["b_i"], np.float32)
    W_h = np.asarray(inputs["W_h"], np.float32)
    b_h = np.asarray(inputs["b_h"], np.float32)
    assert (b_f == 0).all() and (b_i == 0).all() and (b_h == 0).all(), \
        "device program folds zero biases"

    g0 = _g_host(h_0[:, 0, :])  # [B, H]

    x8s, s16s, xh0s, sh0s = [], [], [], []
    for b in range(B):
        xT = np.ascontiguousarray(x[b].T)           # [D, T] f32
        x8 = _pack_x(_q8(xT))
        s16 = _pack_x(_q8(16.0 * (xT - _q8(xT).astype(np.float32))))
        x8s.append(x8)
        s16s.append(s16)
        # contiguous [128, 2, kb, i, 256] copies of the two chunk-0 halves
        xh0s.append(np.ascontiguousarray(
            x8[:, :, :, :TC].reshape(128, NKB, 2, 2, TC // 2)
            .transpose(0, 3, 1, 2, 4)))
        sh0s.append(np.ascontiguousarray(
            s16[:, :, :, :TC].reshape(128, NKB, 2, 2, TC // 2)
            .transpose(0, 3, 1, 2, 4)))

    in_maps = []
    for c in range(NCORES):
        b, hh = divmod(c, 2)
        hs = slice(hh * HS, (hh + 1) * HS)
        Wh = W_h[:, hs]
        A1 = _q8(Wh)
        A2 = _q8(Wh - A1.astype(np.float32))
        Bh = _q8(Wh / 16.0)
        wcat = _pack_w([_q8(W_f[:, hs]), _q8(W_i[:, hs]), A1, Bh, A2])
        auxa = np.ascontiguousarray(
            g0[b, hs].reshape(NHT, 128).T.astype(np.float32))
        in_maps.append({"x8": x8s[b], "s16": s16s[b], "xh0": xh0s[b],
                        "sh0": sh0s[b], "w": wcat, "aux": auxa})

    nc = _get_nc(mm_mode)
    res = run_bass_kernel_spmd(nc, in_maps, core_ids=list(range(NCORES)),
                               trace=trace)

    out = np.empty((B, T + 1, H), np.float32)
    out[:, 0, :] = g0
    for c in range(NCORES):
        b, hh = divmod(c, 2)
        hs = slice(hh * HS, (hh + 1) * HS)
        # device layout [p, ht, t] -> channel (ht*128 + p), time-major
        o = res.results[c]["out"]
        out[b, 1:, hs] = o.transpose(2, 1, 0).reshape(T, HS).astype(np.float32)
    return out, res


def kernel(**inputs):
    out, _ = _run(inputs)
    return out
